# revision 12
# baseline (speedup 1.0000x reference)
"""Distributed Trainium2 Bass kernel for a full causal attention layer.

Problem: B=2, S=2048, D_MODEL=1024, H=16, D_HEAD=64, causal + additive mask.

Sharding (8 cores): data-parallel over batch (cores 0-3 -> batch 0,
cores 4-7 -> batch 1) x tensor-parallel over heads (4 heads per core).
Each core:
  1. projects Q,K (transposed layout [head*dhead, seq]) and V (natural
     layout, with an extra ones-column per head) for its 4 heads,
  2. computes causal attention scores transposed S^T[k,q] = K @ Q^T,
     exp via ScalarE (additive mask folded in as per-partition bias,
     causal mask via a precomputed triangle tile on diagonal blocks,
     upper-triangle blocks skipped entirely),
  3. z_aug^T[65,q] = V_aug^T @ E accumulated over k tiles; row 64 is the
     softmax denominator. Normalizes via reciprocal + K=1 broadcast
     matmul.
  4. AllToAll within its 4-core group to reshard z^T from (all q, local
     heads) to (local 512 q rows, all 16 heads),
  5. output projection for its 512 q rows -> disjoint output row slices.
Host only transposes/shards inputs and concatenates the 8 output slices.
"""

import os
import sys

import numpy as np

for _p in ("/opt/trn_rl_repo", "/root/.axon_site/_ro/trn_rl_repo"):
    if os.path.isdir(_p) and _p not in sys.path:
        sys.path.insert(0, _p)

import concourse.bass as bass  # noqa: E402
import concourse.mybir as mybir  # noqa: E402
from concourse import bacc  # noqa: E402
from concourse import tile  # noqa: E402
from concourse.bass_utils import run_bass_kernel_spmd  # noqa: E402

F32 = mybir.dt.float32
F32R = mybir.dt.float32r

B, S, DM, H, DH = 2, 2048, 1024, 16, 64
N_CORES = 8
GROUP = 4              # cores per batch group
H_LOC = H // GROUP     # heads per core
WCOL = H_LOC * DH      # 256 projected cols per core
QR = S // GROUP        # 512 q rows owned per core after AllToAll
MASK_VAL = -1.0e5
SCALE = 1.0 / np.sqrt(DH).astype(np.float32)

DM_T = DM // 128       # 8 dmodel k-tiles
S_T = S // 128         # 16 seq 128-tiles
S_C = S // 512         # 4 seq 512-chunks


def build_bass():
    nc = bacc.Bacc("TRN2", target_bir_lowering=False, debug=False,
                   num_devices=N_CORES)

    xt_q = nc.dram_tensor("xt_q", [DM, S], F32R, kind="ExternalInput")
    xt_k = nc.dram_tensor("xt_k", [DM, S], F32R, kind="ExternalInput")
    xt_v = nc.dram_tensor("xt_v", [DM, S], F32R, kind="ExternalInput")
    w_q = nc.dram_tensor("w_q", [DM, WCOL], F32R, kind="ExternalInput")
    w_k = nc.dram_tensor("w_k", [DM, WCOL], F32R, kind="ExternalInput")
    w_v = nc.dram_tensor("w_v", [DM, WCOL], F32R, kind="ExternalInput")
    w_o = nc.dram_tensor("w_o", [DM, DM], F32R, kind="ExternalInput")
    bq = nc.dram_tensor("bq", [WCOL, 1], F32, kind="ExternalInput")
    bk = nc.dram_tensor("bk", [WCOL, 1], F32, kind="ExternalInput")
    bvb = nc.dram_tensor("bvb", [128, H_LOC * (DH + 1)], F32R, kind="ExternalInput")
    bob = nc.dram_tensor("bob", [128, DM], F32, kind="ExternalInput")
    maskt = nc.dram_tensor("maskt", [128, S_T], F32, kind="ExternalInput")
    tri = nc.dram_tensor("tri", [128, 896], F32, kind="ExternalInput")
    ones64 = nc.dram_tensor("ones64", [1, DH], F32R, kind="ExternalInput")
    out = nc.dram_tensor("out", [QR, DM], F32, kind="ExternalOutput")

    with tile.TileContext(nc) as tc:
        with (
            tc.tile_pool(name="persist", bufs=1) as pp,
            tc.tile_pool(name="xts", bufs=3) as xtp,
            tc.tile_pool(name="esb", bufs=4) as ep,
            tc.tile_pool(name="work", bufs=2) as wkp,
            tc.tile_pool(name="pa", bufs=4, space="PSUM") as pa,
            tc.tile_pool(name="ps", bufs=2, space="PSUM") as pspool,
            tc.tile_pool(name="pz", bufs=2, space="PSUM") as pzpool,
            tc.tile_pool(name="dram", bufs=1, space="DRAM") as dp,
        ):
            # ---- persistent SBUF tiles ----
            wq_sb = [pp.tile([128, WCOL], F32R, tag=f"wq{i}", name=f"wq{i}") for i in range(DM_T)]
            wk_sb = [pp.tile([128, WCOL], F32R, tag=f"wk{i}", name=f"wk{i}") for i in range(DM_T)]
            wv_sb = [pp.tile([128, WCOL], F32R, tag=f"wv{i}", name=f"wv{i}") for i in range(DM_T)]
            wo_sb = [pp.tile([128, DM], F32R, tag=f"wo{i}", name=f"wo{i}") for i in range(DM_T)]
            qt_sb = [pp.tile([128, S], F32R, tag=f"qt{t}", name=f"qt{t}") for t in range(2)]
            kt_sb = [pp.tile([128, S], F32R, tag=f"kt{t}", name=f"kt{t}") for t in range(2)]
            vaug = [pp.tile([128, H_LOC * (DH + 1)], F32R, tag=f"va{k}", name=f"va{k}")
                    for k in range(S_T)]
            zt_sb = [pp.tile([128, S], F32R, tag=f"zt{t}", name=f"zt{t}") for t in range(2)]
            ztf_e = [pp.tile([128, 256], F32R, tag=f"zfe{i}", name=f"zfe{i}")
                     for i in range(N_CORES)]
            ztf_o = [pp.tile([128, 256], F32R, tag=f"zfo{i}", name=f"zfo{i}")
                     for i in range(N_CORES)]
            bq_sb = [pp.tile([128, 1], F32, tag=f"bq{t}", name=f"bq{t}") for t in range(2)]
            bk_sb = [pp.tile([128, 1], F32, tag=f"bk{t}", name=f"bk{t}") for t in range(2)]
            bvb_sb = pp.tile([128, H_LOC * (DH + 1)], F32R, tag="bvb")
            bob_sb = pp.tile([128, DM], F32, tag="bob")
            maskt_sb = pp.tile([128, S_T], F32, tag="maskt")
            tri_sb = pp.tile([128, 896], F32, tag="tri")
            ones_sb = pp.tile([1, DH], F32R, tag="ones")

            # ---- load constants/weights ----
            for i in range(DM_T):
                nc.sync.dma_start(wq_sb[i], w_q[128 * i:128 * (i + 1), :])
                nc.sync.dma_start(wk_sb[i], w_k[128 * i:128 * (i + 1), :])
                nc.sync.dma_start(wv_sb[i], w_v[128 * i:128 * (i + 1), :])
                nc.sync.dma_start(wo_sb[i], w_o[128 * i:128 * (i + 1), :])
            for t in range(2):
                nc.sync.dma_start(bq_sb[t], bq[128 * t:128 * (t + 1), :])
                nc.sync.dma_start(bk_sb[t], bk[128 * t:128 * (t + 1), :])
            nc.sync.dma_start(bvb_sb, bvb[:, :])
            nc.sync.dma_start(bob_sb, bob[:, :])
            nc.sync.dma_start(maskt_sb, maskt[:, :])
            nc.sync.dma_start(tri_sb, tri[:, :])
            nc.sync.dma_start(ones_sb, ones64[:, :])

            # ---- Q/K projections (transposed layout) ----
            # QT[wcol, x] = sum_dm W_Q[dm, wcol] * X[x, dm]
            for xc in range(2):          # 1024-col chunks
                for src_dram, w_t, b_t, dst in (
                    (xt_q, wq_sb, bq_sb, qt_sb),
                    (xt_k, wk_sb, bk_sb, kt_sb),
                ):
                    xx = [xtp.tile([128, 1024], F32R, tag="xq", name="xq")
                          for _ in range(DM_T)]
                    pq = [[pa.tile([128, 512], F32, tag="pa", name="pa")
                           for _ in range(2)] for _ in range(2)]  # [wc][half]
                    for dm in range(DM_T):
                        nc.sync.dma_start(
                            xx[dm],
                            src_dram[128 * dm:128 * (dm + 1),
                                     1024 * xc:1024 * (xc + 1)])
                        for wc in range(2):
                            for hf in range(2):
                                nc.tensor.matmul(
                                    pq[wc][hf],
                                    w_t[dm][:, 128 * wc:128 * (wc + 1)],
                                    xx[dm][:, 512 * hf:512 * (hf + 1)],
                                    start=(dm == 0), stop=(dm == DM_T - 1))
                    for wc in range(2):
                        for hf in range(2):
                            o = 1024 * xc + 512 * hf
                            nc.vector.tensor_scalar_add(
                                dst[wc][:, o:o + 512], pq[wc][hf], b_t[wc])

            # ---- V projection (natural layout + ones column per head) ----
            for xc in range(S_C):
                xv_t = [xtp.tile([128, 512], F32R, tag="xv", name="xv")
                        for _ in range(DM_T)]
                psv = [pa.tile([128, WCOL], F32, tag="pa", name="pav")
                       for _ in range(4)]
                for dm in range(DM_T):
                    nc.sync.dma_start(
                        xv_t[dm],
                        xt_v[128 * dm:128 * (dm + 1), 512 * xc:512 * (xc + 1)])
                    for x4 in range(4):
                        nc.tensor.matmul(
                            psv[x4], xv_t[dm][:, 128 * x4:128 * (x4 + 1)],
                            wv_sb[dm], start=(dm == 0), stop=(dm == DM_T - 1))
                for x4 in range(4):
                    ki = 4 * xc + x4
                    va3 = vaug[ki].rearrange("p (h x) -> p h x", h=H_LOC)
                    bvb3 = bvb_sb.rearrange("p (h x) -> p h x", h=H_LOC)
                    nc.vector.scalar_tensor_tensor(
                        va3[:, :, 0:DH],
                        psv[x4].rearrange("p (h d) -> p h d", h=H_LOC),
                        1.0, bvb3[:, :, 0:DH],
                        op0=mybir.AluOpType.mult, op1=mybir.AluOpType.add)
                    nc.vector.tensor_copy(
                        va3[:, :, DH:DH + 1], bvb3[:, :, DH:DH + 1])

            # ---- causal attention, scores transposed [k, q] ----
            # AllToAll is split in two: heads {0,1} (zt tile 0) fire while
            # heads {2,3} still compute.
            a2a_in = [dp.tile([N_CORES * 128, 256], F32R, tag=f"a2a_in{t}",
                              name=f"a2a_in{t}") for t in range(2)]
            a2a_out = [dp.tile([N_CORES * 128, 256], F32R, tag=f"a2a_out{t}",
                               name=f"a2a_out{t}") for t in range(2)]
            for h in range(H_LOC):
                th, ho = h // 2, 64 * (h % 2)
                for c in range(S_C):
                    kmax = 4 * c + 4  # k tiles 0..kmax-1 (rest fully masked)
                    psz = pzpool.tile([DH + 1, 512], F32, tag="pz")
                    for ki in range(kmax):
                        pss = pspool.tile([128, 512], F32, tag="ps")
                        nc.tensor.matmul(
                            pss,
                            kt_sb[th][ho:ho + DH, 128 * ki:128 * (ki + 1)],
                            qt_sb[th][ho:ho + DH, 512 * c:512 * (c + 1)],
                            start=True, stop=True)
                        j = ki - 4 * c
                        if j >= 0:  # diagonal: mask cols [0, 128(j+1))
                            w = 128 * (j + 1)
                            nc.vector.tensor_add(
                                pss[:, 0:w], pss[:, 0:w],
                                tri_sb[:, 384 - 128 * j:384 - 128 * j + w])
                        esb = ep.tile([128, 512], F32R, tag="e")
                        nc.scalar.activation(
                            esb, pss, mybir.ActivationFunctionType.Exp,
                            bias=maskt_sb[:, ki:ki + 1], scale=float(SCALE))
                        nc.tensor.matmul(
                            psz, vaug[ki][:, (DH + 1) * h:(DH + 1) * (h + 1)],
                            esb, start=(ki == 0), stop=(ki == kmax - 1))
                    # normalize: recip of denominator row, broadcast via K=1 mm
                    recip = wkp.tile([1, 512], F32R, tag="recip")
                    with nc.allow_low_precision(reason="f32r softmax denom"):
                        nc.vector.reciprocal(recip, psz[DH:DH + 1, :])
                    psb = pspool.tile([DH, 512], F32, tag="ps", name="psb")
                    nc.tensor.matmul(psb, ones_sb, recip, start=True, stop=True)
                    zraw = wkp.tile([DH, 512], F32, tag="zraw")
                    nc.vector.tensor_copy(zraw, psz[0:DH, :])
                    nc.vector.tensor_mul(
                        zt_sb[th][ho:ho + DH, 512 * c:512 * (c + 1)], zraw, psb)
                if h % 2 == 1:
                    # zt tile th complete for all q: fire its AllToAll.
                    # My shard j = my 2 heads' z^T for q cols [256j, 256j+256)
                    # of my batch; received slot p = peer p's heads
                    # {4p+2t, 4p+2t+1} for my 256 q rows.
                    for j in range(N_CORES):
                        nc.sync.dma_start(
                            a2a_in[th][128 * j:128 * (j + 1), :],
                            zt_sb[th][:, 256 * j:256 * (j + 1)])
                    nc.gpsimd.collective_compute(
                        "AllToAll", mybir.AluOpType.bypass,
                        replica_groups=[[0, 1, 2, 3, 4, 5, 6, 7]],
                        ins=[a2a_in[th].opt()], outs=[a2a_out[th].opt()])
                    dst = ztf_e if th == 0 else ztf_o
                    for p in range(N_CORES):
                        nc.sync.dma_start(
                            dst[p], a2a_out[th][128 * p:128 * (p + 1), :])

            # ---- output projection: 256 q rows per batch ----
            # ztf_e[p] holds global heads {4p, 4p+1} -> W_O tile 2(p%4);
            # ztf_o[p] holds heads {4p+2, 4p+3} -> W_O tile 2(p%4)+1.
            for bh in range(2):
                for qt in range(2):
                    osb = wkp.tile([128, DM], F32, tag="osb")
                    for mc in range(2):
                        pso = pa.tile([128, 512], F32, tag="pa", name="pso")
                        for g in range(4):
                            nc.tensor.matmul(
                                pso,
                                ztf_e[4 * bh + g][:, 128 * qt:128 * (qt + 1)],
                                wo_sb[2 * g][:, 512 * mc:512 * (mc + 1)],
                                start=(g == 0), stop=False)
                        for g in range(4):
                            nc.tensor.matmul(
                                pso,
                                ztf_o[4 * bh + g][:, 128 * qt:128 * (qt + 1)],
                                wo_sb[2 * g + 1][:, 512 * mc:512 * (mc + 1)],
                                start=False, stop=(g == 3))
                        nc.vector.tensor_add(
                            osb[:, 512 * mc:512 * (mc + 1)], pso,
                            bob_sb[:, 512 * mc:512 * (mc + 1)])
                    nc.sync.dma_start(
                        out[256 * bh + 128 * qt:256 * bh + 128 * (qt + 1), :], osb)

    nc.finalize()
    return nc


_NC = None


def _get_nc():
    global _NC
    if _NC is None:
        _NC = build_bass()
    return _NC


def make_in_maps(query_input, key_input, value_input, additive_attention_mask,
                 W_Q, W_K, W_V, W_O, b_Q, b_K, b_V, b_O):
    f = np.float32
    tri = np.where(
        np.arange(896, dtype=np.int64)[None, :] - 384
        >= np.arange(128, dtype=np.int64)[:, None],
        f(0.0), f(MASK_VAL)).astype(f)
    bob = np.ascontiguousarray(np.broadcast_to(b_O.astype(f), (128, DM)))
    wo = np.ascontiguousarray(W_O.astype(f).reshape(DM, DM))
    in_maps = []
    for c in range(N_CORES):
        b, rk = c // GROUP, c % GROUP
        hs = slice(H_LOC * rk, H_LOC * (rk + 1))
        wq = np.ascontiguousarray(
            W_Q[hs].astype(f).transpose(1, 0, 2).reshape(DM, WCOL))
        wk = np.ascontiguousarray(
            W_K[hs].astype(f).transpose(1, 0, 2).reshape(DM, WCOL))
        wv = np.ascontiguousarray(
            W_V[hs].astype(f).transpose(1, 0, 2).reshape(DM, WCOL))
        bvb = np.zeros((128, H_LOC * (DH + 1)), f)
        for h in range(H_LOC):
            bvb[:, (DH + 1) * h:(DH + 1) * h + DH] = b_V[H_LOC * rk + h].astype(f)
            bvb[:, (DH + 1) * h + DH] = 1.0
        in_maps.append({
            "xt_q": np.ascontiguousarray(query_input[b].astype(f).T),
            "xt_k": np.ascontiguousarray(key_input[b].astype(f).T),
            "xt_v": np.ascontiguousarray(value_input[b].astype(f).T),
            "w_q": wq, "w_k": wk, "w_v": wv, "w_o": wo,
            "bq": np.ascontiguousarray(b_Q[hs].astype(f).reshape(WCOL, 1)),
            "bk": np.ascontiguousarray(b_K[hs].astype(f).reshape(WCOL, 1)),
            "bvb": bvb, "bob": bob,
            "ones64": np.ones((1, DH), f),
            "maskt": np.ascontiguousarray(
                additive_attention_mask[b, 0, 0].astype(f).reshape(S_T, 128).T),
            "tri": tri,
        })
    return in_maps


def assemble_output(results):
    out = np.empty((B, S, DM), np.float32)
    for c in range(N_CORES):
        out[0, 256 * c:256 * (c + 1), :] = results[c]["out"][:256]
        out[1, 256 * c:256 * (c + 1), :] = results[c]["out"][256:]
    return out


def kernel(**inputs):
    nc = _get_nc()
    in_maps = make_in_maps(**inputs)
    res = run_bass_kernel_spmd(nc, in_maps, core_ids=list(range(N_CORES)))
    return assemble_output(res.results)


# revision 14
# speedup vs baseline: 1.4161x; 1.4161x over previous
"""Distributed Trainium2 Bass kernel for a full causal attention layer.

Problem: B=2, S=2048, D_MODEL=1024, H=16, D_HEAD=64, causal + additive mask.

Sharding (8 cores): data-parallel over batch (cores 0-3 -> batch 0,
cores 4-7 -> batch 1) x tensor-parallel over heads (4 heads per core).
Each core:
  1. projects Q,K (transposed layout [head*dhead, seq]) and V (natural
     layout, with an extra ones-column per head) for its 4 heads,
  2. computes causal attention scores transposed S^T[k,q] = K @ Q^T,
     exp via ScalarE (additive mask folded in as per-partition bias,
     causal mask via a precomputed triangle tile on diagonal blocks,
     upper-triangle blocks skipped entirely),
  3. z_aug^T[65,q] = V_aug^T @ E accumulated over k tiles; row 64 is the
     softmax denominator. Normalizes via reciprocal + K=1 broadcast
     matmul.
  4. AllToAll within its 4-core group to reshard z^T from (all q, local
     heads) to (local 512 q rows, all 16 heads),
  5. output projection for its 512 q rows -> disjoint output row slices.
Host only transposes/shards inputs and concatenates the 8 output slices.
"""

import os
import sys

import ml_dtypes
import numpy as np

for _p in ("/opt/trn_rl_repo", "/root/.axon_site/_ro/trn_rl_repo"):
    if os.path.isdir(_p) and _p not in sys.path:
        sys.path.insert(0, _p)

import concourse.bass as bass  # noqa: E402
import concourse.mybir as mybir  # noqa: E402
from concourse import bacc  # noqa: E402
from concourse import tile  # noqa: E402
from concourse.bass_utils import run_bass_kernel_spmd  # noqa: E402

F32 = mybir.dt.float32
F32R = mybir.dt.float32r
BF16 = mybir.dt.bfloat16

B, S, DM, H, DH = 2, 2048, 1024, 16, 64
N_CORES = 8
GROUP = 4              # cores per batch group
H_LOC = H // GROUP     # heads per core
WCOL = H_LOC * DH      # 256 projected cols per core
QR = S // GROUP        # 512 q rows owned per core after AllToAll
MASK_VAL = -1.0e5
SCALE = 1.0 / np.sqrt(DH).astype(np.float32)

DM_T = DM // 128       # 8 dmodel k-tiles
S_T = S // 128         # 16 seq 128-tiles
S_C = S // 512         # 4 seq 512-chunks


def build_bass():
    nc = bacc.Bacc("TRN2", target_bir_lowering=False, debug=False,
                   num_devices=N_CORES)

    xt_q = nc.dram_tensor("xt_q", [DM, S], BF16, kind="ExternalInput")
    xt_k = nc.dram_tensor("xt_k", [DM, S], BF16, kind="ExternalInput")
    xt_v = nc.dram_tensor("xt_v", [DM, S], BF16, kind="ExternalInput")
    w_q = nc.dram_tensor("w_q", [DM, WCOL], BF16, kind="ExternalInput")
    w_k = nc.dram_tensor("w_k", [DM, WCOL], BF16, kind="ExternalInput")
    w_v = nc.dram_tensor("w_v", [DM, WCOL], BF16, kind="ExternalInput")
    w_o = nc.dram_tensor("w_o", [DM, DM], BF16, kind="ExternalInput")
    bq = nc.dram_tensor("bq", [WCOL, 1], F32, kind="ExternalInput")
    bk = nc.dram_tensor("bk", [WCOL, 1], F32, kind="ExternalInput")
    bvb = nc.dram_tensor("bvb", [128, H_LOC * (DH + 1)], BF16, kind="ExternalInput")
    bob = nc.dram_tensor("bob", [128, DM], F32, kind="ExternalInput")
    maskt = nc.dram_tensor("maskt", [128, S_T], F32, kind="ExternalInput")
    tri = nc.dram_tensor("tri", [128, 896], F32, kind="ExternalInput")
    sel = nc.dram_tensor("sel", [8, 512], BF16, kind="ExternalInput")
    out = nc.dram_tensor("out", [QR, DM], F32, kind="ExternalOutput")

    with tile.TileContext(nc) as tc:
        with (
            tc.tile_pool(name="persist", bufs=1) as pp,
            tc.tile_pool(name="xts", bufs=3) as xtp,
            tc.tile_pool(name="esb", bufs=4) as ep,
            tc.tile_pool(name="work", bufs=2) as wkp,
            tc.tile_pool(name="pa", bufs=4, space="PSUM") as pa,
            tc.tile_pool(name="ps", bufs=2, space="PSUM") as pspool,
            tc.tile_pool(name="pz", bufs=2, space="PSUM") as pzpool,
            tc.tile_pool(name="dram", bufs=1, space="DRAM") as dp,
        ):
            # ---- persistent SBUF tiles ----
            wq_sb = [pp.tile([128, WCOL], BF16, tag=f"wq{i}", name=f"wq{i}") for i in range(DM_T)]
            wk_sb = [pp.tile([128, WCOL], BF16, tag=f"wk{i}", name=f"wk{i}") for i in range(DM_T)]
            wv_sb = [pp.tile([128, WCOL], BF16, tag=f"wv{i}", name=f"wv{i}") for i in range(DM_T)]
            wo_sb = [pp.tile([128, DM], BF16, tag=f"wo{i}", name=f"wo{i}") for i in range(DM_T)]
            qt_sb = [pp.tile([128, S], BF16, tag=f"qt{t}", name=f"qt{t}") for t in range(2)]
            kt_sb = [pp.tile([128, S], BF16, tag=f"kt{t}", name=f"kt{t}") for t in range(2)]
            vaug = [pp.tile([128, H_LOC * (DH + 1)], BF16, tag=f"va{k}", name=f"va{k}")
                    for k in range(S_T)]
            zt_sb = [pp.tile([128, S], BF16, tag=f"zt{t}", name=f"zt{t}") for t in range(2)]
            ztf_e = [pp.tile([128, 256], BF16, tag=f"zfe{i}", name=f"zfe{i}")
                     for i in range(N_CORES)]
            ztf_o = [pp.tile([128, 256], BF16, tag=f"zfo{i}", name=f"zfo{i}")
                     for i in range(N_CORES)]
            bq_sb = [pp.tile([128, 1], F32, tag=f"bq{t}", name=f"bq{t}") for t in range(2)]
            bk_sb = [pp.tile([128, 1], F32, tag=f"bk{t}", name=f"bk{t}") for t in range(2)]
            bvb_sb = pp.tile([128, H_LOC * (DH + 1)], BF16, tag="bvb")
            bob_sb = pp.tile([128, DM], F32, tag="bob")
            maskt_sb = pp.tile([128, S_T], F32, tag="maskt")
            tri_sb = pp.tile([128, 896], F32, tag="tri")
            sel_sb = pp.tile([8, 512], BF16, tag="sel")

            # ---- load constants/weights ----
            for i in range(DM_T):
                nc.sync.dma_start(wq_sb[i], w_q[128 * i:128 * (i + 1), :])
                nc.sync.dma_start(wk_sb[i], w_k[128 * i:128 * (i + 1), :])
                nc.sync.dma_start(wv_sb[i], w_v[128 * i:128 * (i + 1), :])
                nc.sync.dma_start(wo_sb[i], w_o[128 * i:128 * (i + 1), :])
            for t in range(2):
                nc.sync.dma_start(bq_sb[t], bq[128 * t:128 * (t + 1), :])
                nc.sync.dma_start(bk_sb[t], bk[128 * t:128 * (t + 1), :])
            nc.sync.dma_start(bvb_sb, bvb[:, :])
            nc.sync.dma_start(bob_sb, bob[:, :])
            nc.sync.dma_start(maskt_sb, maskt[:, :])
            nc.sync.dma_start(tri_sb, tri[:, :])
            nc.sync.dma_start(sel_sb, sel[:, :])

            # ---- Q/K projections (transposed layout) ----
            # QT[wcol, x] = sum_dm W_Q[dm, wcol] * X[x, dm]
            for xc in range(2):          # 1024-col chunks
                for src_dram, w_t, b_t, dst in (
                    (xt_q, wq_sb, bq_sb, qt_sb),
                    (xt_k, wk_sb, bk_sb, kt_sb),
                ):
                    xx = [xtp.tile([128, 1024], BF16, tag="xq", name="xq")
                          for _ in range(DM_T)]
                    pq = [[pa.tile([128, 512], F32, tag="pa", name="pa")
                           for _ in range(2)] for _ in range(2)]  # [wc][half]
                    for dm in range(DM_T):
                        nc.sync.dma_start(
                            xx[dm],
                            src_dram[128 * dm:128 * (dm + 1),
                                     1024 * xc:1024 * (xc + 1)])
                        for wc in range(2):
                            for hf in range(2):
                                nc.tensor.matmul(
                                    pq[wc][hf],
                                    w_t[dm][:, 128 * wc:128 * (wc + 1)],
                                    xx[dm][:, 512 * hf:512 * (hf + 1)],
                                    start=(dm == 0), stop=(dm == DM_T - 1))
                    for wc in range(2):
                        for hf in range(2):
                            o = 1024 * xc + 512 * hf
                            nc.vector.tensor_scalar_add(
                                dst[wc][:, o:o + 512], pq[wc][hf], b_t[wc])

            # ---- V projection (natural layout + ones column per head) ----
            for xc in range(S_C):
                xv_t = [xtp.tile([128, 512], BF16, tag="xv", name="xv")
                        for _ in range(DM_T)]
                psv = [pa.tile([128, WCOL], F32, tag="pa", name="pav")
                       for _ in range(4)]
                for dm in range(DM_T):
                    nc.sync.dma_start(
                        xv_t[dm],
                        xt_v[128 * dm:128 * (dm + 1), 512 * xc:512 * (xc + 1)])
                    for x4 in range(4):
                        nc.tensor.matmul(
                            psv[x4], xv_t[dm][:, 128 * x4:128 * (x4 + 1)],
                            wv_sb[dm], start=(dm == 0), stop=(dm == DM_T - 1))
                for x4 in range(4):
                    ki = 4 * xc + x4
                    va3 = vaug[ki].rearrange("p (h x) -> p h x", h=H_LOC)
                    bvb3 = bvb_sb.rearrange("p (h x) -> p h x", h=H_LOC)
                    nc.vector.scalar_tensor_tensor(
                        va3[:, :, 0:DH],
                        psv[x4].rearrange("p (h d) -> p h d", h=H_LOC),
                        1.0, bvb3[:, :, 0:DH],
                        op0=mybir.AluOpType.mult, op1=mybir.AluOpType.add)
                    nc.vector.tensor_copy(
                        va3[:, :, DH:DH + 1], bvb3[:, :, DH:DH + 1])

            # ---- causal attention, scores transposed [k, q] ----
            # AllToAll is split in two: heads {0,1} (zt tile 0) fire while
            # heads {2,3} still compute.
            a2a_in = [dp.tile([N_CORES * 128, 256], BF16, tag=f"a2a_in{t}",
                              name=f"a2a_in{t}") for t in range(2)]
            a2a_out = [dp.tile([N_CORES * 128, 256], BF16, tag=f"a2a_out{t}",
                               name=f"a2a_out{t}") for t in range(2)]
            zaug = [ep.tile([DH + 1, 512], BF16, tag=f"zaug{i}", name=f"zaug{i}",
                            bufs=1) for i in range(16)]
            se_sb = pp.tile([8, 512], BF16, tag="se")
            for h in range(H_LOC):
                th, ho = h // 2, 64 * (h % 2)
                for c in range(S_C):
                    kmax = 4 * c + 4  # k tiles 0..kmax-1 (rest fully masked)
                    psz = pzpool.tile([DH + 1, 512], F32, tag="pz")
                    for ki in range(kmax):
                        pss = pspool.tile([128, 512], F32, tag="ps")
                        nc.tensor.matmul(
                            pss,
                            kt_sb[th][ho:ho + DH, 128 * ki:128 * (ki + 1)],
                            qt_sb[th][ho:ho + DH, 512 * c:512 * (c + 1)],
                            start=True, stop=True)
                        j = ki - 4 * c
                        if j >= 0:  # diagonal: mask cols [0, 128(j+1))
                            w = 128 * (j + 1)
                            nc.vector.tensor_add(
                                pss[:, 0:w], pss[:, 0:w],
                                tri_sb[:, 384 - 128 * j:384 - 128 * j + w])
                        esb = ep.tile([128, 512], BF16, tag="e")
                        nc.scalar.activation(
                            esb, pss, mybir.ActivationFunctionType.Exp,
                            bias=maskt_sb[:, ki:ki + 1], scale=float(SCALE))
                        nc.tensor.matmul(
                            psz, vaug[ki][:, (DH + 1) * h:(DH + 1) * (h + 1)],
                            esb, start=(ki == 0), stop=(ki == kmax - 1))
                    # stash unnormalized z + denominator; normalize in a
                    # batched pass per zt tile (single wide reciprocal)
                    za = zaug[8 * (h % 2) + c]
                    with nc.allow_low_precision(reason="bf16 attention"):
                        nc.vector.tensor_copy(za, psz)
                    nc.sync.dma_start(se_sb[4 * (h % 2) + c:4 * (h % 2) + c + 1, :],
                                      za[DH:DH + 1, :])
                if h % 2 == 1:
                    ser = wkp.tile([8, 512], BF16, tag="ser")
                    with nc.allow_low_precision(reason="bf16 attention"):
                        nc.vector.reciprocal(ser, se_sb)
                    for h2 in range(2):
                        ho2 = 64 * h2
                        for c in range(S_C):
                            rsel = 4 * h2 + c
                            psb = pspool.tile([DH, 512], F32, tag="ps", name="psb")
                            nc.tensor.matmul(psb, sel_sb[:, DH * rsel:DH * (rsel + 1)],
                                             ser, start=True, stop=True)
                            with nc.allow_low_precision(reason="bf16 attention"):
                                nc.vector.tensor_mul(
                                    zt_sb[th][ho2:ho2 + DH, 512 * c:512 * (c + 1)],
                                    zaug[8 * h2 + c][0:DH, :], psb)
                    # zt tile th complete for all q: fire its AllToAll.
                    # My shard j = my 2 heads' z^T for q cols [256j, 256j+256)
                    # of my batch; received slot p = peer p's heads
                    # {4p+2t, 4p+2t+1} for my 256 q rows.
                    for j in range(N_CORES):
                        nc.sync.dma_start(
                            a2a_in[th][128 * j:128 * (j + 1), :],
                            zt_sb[th][:, 256 * j:256 * (j + 1)])
                    nc.gpsimd.collective_compute(
                        "AllToAll", mybir.AluOpType.bypass,
                        replica_groups=[[0, 1, 2, 3, 4, 5, 6, 7]],
                        ins=[a2a_in[th].opt()], outs=[a2a_out[th].opt()])
                    dst = ztf_e if th == 0 else ztf_o
                    for p in range(N_CORES):
                        nc.sync.dma_start(
                            dst[p], a2a_out[th][128 * p:128 * (p + 1), :])

            # ---- output projection: 256 q rows per batch ----
            # ztf_e[p] holds global heads {4p, 4p+1} -> W_O tile 2(p%4);
            # ztf_o[p] holds heads {4p+2, 4p+3} -> W_O tile 2(p%4)+1.
            for bh in range(2):
                for qt in range(2):
                    osb = wkp.tile([128, DM], F32, tag="osb")
                    for mc in range(2):
                        pso = pa.tile([128, 512], F32, tag="pa", name="pso")
                        for g in range(4):
                            nc.tensor.matmul(
                                pso,
                                ztf_e[4 * bh + g][:, 128 * qt:128 * (qt + 1)],
                                wo_sb[2 * g][:, 512 * mc:512 * (mc + 1)],
                                start=(g == 0), stop=False)
                        for g in range(4):
                            nc.tensor.matmul(
                                pso,
                                ztf_o[4 * bh + g][:, 128 * qt:128 * (qt + 1)],
                                wo_sb[2 * g + 1][:, 512 * mc:512 * (mc + 1)],
                                start=False, stop=(g == 3))
                        nc.vector.tensor_add(
                            osb[:, 512 * mc:512 * (mc + 1)], pso,
                            bob_sb[:, 512 * mc:512 * (mc + 1)])
                    nc.sync.dma_start(
                        out[256 * bh + 128 * qt:256 * bh + 128 * (qt + 1), :], osb)

    nc.finalize()
    return nc


_NC = None


def _get_nc():
    global _NC
    if _NC is None:
        _NC = build_bass()
    return _NC


def make_in_maps(query_input, key_input, value_input, additive_attention_mask,
                 W_Q, W_K, W_V, W_O, b_Q, b_K, b_V, b_O):
    f = np.float32
    bf = ml_dtypes.bfloat16
    tri = np.where(
        np.arange(896, dtype=np.int64)[None, :] - 384
        >= np.arange(128, dtype=np.int64)[:, None],
        f(0.0), f(MASK_VAL)).astype(f)
    bob = np.ascontiguousarray(np.broadcast_to(b_O.astype(f), (128, DM)))
    sel_host = np.zeros((8, 512), ml_dtypes.bfloat16)
    for rr in range(8):
        sel_host[rr, DH * rr:DH * (rr + 1)] = 1.0
    wo = np.ascontiguousarray(W_O.astype(f).reshape(DM, DM)).astype(bf)
    in_maps = []
    for c in range(N_CORES):
        b, rk = c // GROUP, c % GROUP
        hs = slice(H_LOC * rk, H_LOC * (rk + 1))
        wq = np.ascontiguousarray(
            W_Q[hs].astype(f).transpose(1, 0, 2).reshape(DM, WCOL)).astype(bf)
        wk = np.ascontiguousarray(
            W_K[hs].astype(f).transpose(1, 0, 2).reshape(DM, WCOL)).astype(bf)
        wv = np.ascontiguousarray(
            W_V[hs].astype(f).transpose(1, 0, 2).reshape(DM, WCOL)).astype(bf)
        bvb = np.zeros((128, H_LOC * (DH + 1)), ml_dtypes.bfloat16)
        for h in range(H_LOC):
            bvb[:, (DH + 1) * h:(DH + 1) * h + DH] = b_V[H_LOC * rk + h].astype(f)
            bvb[:, (DH + 1) * h + DH] = 1.0
        in_maps.append({
            "xt_q": np.ascontiguousarray(query_input[b].astype(f).T).astype(bf),
            "xt_k": np.ascontiguousarray(key_input[b].astype(f).T).astype(bf),
            "xt_v": np.ascontiguousarray(value_input[b].astype(f).T).astype(bf),
            "w_q": wq, "w_k": wk, "w_v": wv, "w_o": wo,
            "bq": np.ascontiguousarray(b_Q[hs].astype(f).reshape(WCOL, 1)),
            "bk": np.ascontiguousarray(b_K[hs].astype(f).reshape(WCOL, 1)),
            "bvb": bvb, "bob": bob,
            "sel": sel_host,
            "maskt": np.ascontiguousarray(
                additive_attention_mask[b, 0, 0].astype(f).reshape(S_T, 128).T),
            "tri": tri,
        })
    return in_maps


def assemble_output(results):
    out = np.empty((B, S, DM), np.float32)
    for c in range(N_CORES):
        out[0, 256 * c:256 * (c + 1), :] = results[c]["out"][:256]
        out[1, 256 * c:256 * (c + 1), :] = results[c]["out"][256:]
    return out


def kernel(**inputs):
    nc = _get_nc()
    in_maps = make_in_maps(**inputs)
    res = run_bass_kernel_spmd(nc, in_maps, core_ids=list(range(N_CORES)))
    return assemble_output(res.results)


# revision 17
# speedup vs baseline: 1.5328x; 1.0824x over previous
"""Distributed Trainium2 Bass kernel for a full causal attention layer.

Problem: B=2, S=2048, D_MODEL=1024, H=16, D_HEAD=64, causal + additive mask.

Sharding (8 cores): data-parallel over batch (cores 0-3 -> batch 0,
cores 4-7 -> batch 1) x tensor-parallel over heads (4 heads per core).
Each core:
  1. projects Q,K (transposed layout [head*dhead, seq]) and V (natural
     layout, with an extra ones-column per head) for its 4 heads,
  2. computes causal attention scores transposed S^T[k,q] = K @ Q^T,
     exp via ScalarE (additive mask folded in as per-partition bias,
     causal mask via a precomputed triangle tile on diagonal blocks,
     upper-triangle blocks skipped entirely),
  3. z_aug^T[65,q] = V_aug^T @ E accumulated over k tiles; row 64 is the
     softmax denominator. Normalizes via reciprocal + K=1 broadcast
     matmul.
  4. AllToAll within its 4-core group to reshard z^T from (all q, local
     heads) to (local 512 q rows, all 16 heads),
  5. output projection for its 512 q rows -> disjoint output row slices.
Host only transposes/shards inputs and concatenates the 8 output slices.
"""

import os
import sys

import ml_dtypes
import numpy as np

for _p in ("/opt/trn_rl_repo", "/root/.axon_site/_ro/trn_rl_repo"):
    if os.path.isdir(_p) and _p not in sys.path:
        sys.path.insert(0, _p)

import concourse.bass as bass  # noqa: E402
import concourse.mybir as mybir  # noqa: E402
from concourse import bacc  # noqa: E402
from concourse import tile  # noqa: E402
from concourse.bass_utils import run_bass_kernel_spmd  # noqa: E402

F32 = mybir.dt.float32
F32R = mybir.dt.float32r
BF16 = mybir.dt.bfloat16

B, S, DM, H, DH = 2, 2048, 1024, 16, 64
N_CORES = 8
GROUP = 4              # cores per batch group
H_LOC = H // GROUP     # heads per core
WCOL = H_LOC * DH      # 256 projected cols per core
QR = S // GROUP        # 512 q rows owned per core after AllToAll
MASK_VAL = -1.0e5
SCALE = 1.0 / np.sqrt(DH).astype(np.float32)

DM_T = DM // 128       # 8 dmodel k-tiles
S_T = S // 128         # 16 seq 128-tiles
S_C = S // 512         # 4 seq 512-chunks


def build_bass():
    nc = bacc.Bacc("TRN2", target_bir_lowering=False, debug=False,
                   num_devices=N_CORES)

    xt_q = nc.dram_tensor("xt_q", [DM, S], BF16, kind="ExternalInput")
    xt_k = nc.dram_tensor("xt_k", [DM, S], BF16, kind="ExternalInput")
    xt_v = nc.dram_tensor("xt_v", [DM, S], BF16, kind="ExternalInput")
    w_q = nc.dram_tensor("w_q", [DM, WCOL], BF16, kind="ExternalInput")
    w_k = nc.dram_tensor("w_k", [DM, WCOL], BF16, kind="ExternalInput")
    w_v = nc.dram_tensor("w_v", [DM, WCOL], BF16, kind="ExternalInput")
    w_o = nc.dram_tensor("w_o", [DM, DM], BF16, kind="ExternalInput")
    bq = nc.dram_tensor("bq", [WCOL, 1], F32, kind="ExternalInput")
    bk = nc.dram_tensor("bk", [WCOL, 1], F32, kind="ExternalInput")
    bvb = nc.dram_tensor("bvb", [128, H_LOC * (DH + 1)], BF16, kind="ExternalInput")
    bob = nc.dram_tensor("bob", [128, DM], F32, kind="ExternalInput")
    maskt = nc.dram_tensor("maskt", [128, S_T], F32, kind="ExternalInput")
    tri = nc.dram_tensor("tri", [128, 128], F32, kind="ExternalInput")
    sel = nc.dram_tensor("sel", [4, 256], BF16, kind="ExternalInput")
    out = nc.dram_tensor("out", [QR, DM], F32, kind="ExternalOutput")

    with tile.TileContext(nc) as tc:
        with (
            tc.tile_pool(name="persist", bufs=1) as pp,
            tc.tile_pool(name="xts", bufs=8) as xtp,
            tc.tile_pool(name="esb", bufs=6) as ep,
            tc.tile_pool(name="work", bufs=2) as wkp,
            tc.tile_pool(name="pa", bufs=2, space="PSUM") as pa,
            tc.tile_pool(name="ps", bufs=2, space="PSUM") as pspool,
            tc.tile_pool(name="dram", bufs=1, space="DRAM") as dp,
        ):
            # ---- persistent SBUF tiles ----
            wq_sb = [pp.tile([128, WCOL], BF16, tag=f"wq{i}", name=f"wq{i}") for i in range(DM_T)]
            wk_sb = [pp.tile([128, WCOL], BF16, tag=f"wk{i}", name=f"wk{i}") for i in range(DM_T)]
            wv_sb = [pp.tile([128, WCOL], BF16, tag=f"wv{i}", name=f"wv{i}") for i in range(DM_T)]
            wo_sb = [pp.tile([128, DM], BF16, tag=f"wo{i}", name=f"wo{i}") for i in range(DM_T)]
            qt_sb = [pp.tile([128, S], BF16, tag=f"qt{t}", name=f"qt{t}") for t in range(2)]
            kt_sb = [pp.tile([128, S], BF16, tag=f"kt{t}", name=f"kt{t}") for t in range(2)]
            vaug = [pp.tile([128, H_LOC * (DH + 1)], BF16, tag=f"va{k}", name=f"va{k}")
                    for k in range(S_T)]
            zt_sb = [pp.tile([128, S], BF16, tag=f"zt{t}", name=f"zt{t}") for t in range(2)]
            ztf_e = [pp.tile([128, 256], BF16, tag=f"zfe{i}", name=f"zfe{i}")
                     for i in range(N_CORES)]
            ztf_o = [pp.tile([128, 256], BF16, tag=f"zfo{i}", name=f"zfo{i}")
                     for i in range(N_CORES)]
            bq_sb = [pp.tile([128, 1], F32, tag=f"bq{t}", name=f"bq{t}") for t in range(2)]
            bk_sb = [pp.tile([128, 1], F32, tag=f"bk{t}", name=f"bk{t}") for t in range(2)]
            bvb_sb = pp.tile([128, H_LOC * (DH + 1)], BF16, tag="bvb")
            bob_sb = pp.tile([128, DM], F32, tag="bob")
            maskt_sb = pp.tile([128, S_T], F32, tag="maskt")
            tri_sb = pp.tile([128, 128], F32, tag="tri")
            sel_sb = pp.tile([4, 256], BF16, tag="sel")
            se_sb = pp.tile([4, 1024], BF16, tag="se")

            # ---- load constants + Q/K/V weights (W_O deferred) ----
            for i in range(DM_T):
                nc.sync.dma_start(wq_sb[i], w_q[128 * i:128 * (i + 1), :])
                nc.sync.dma_start(wk_sb[i], w_k[128 * i:128 * (i + 1), :])
                nc.sync.dma_start(wv_sb[i], w_v[128 * i:128 * (i + 1), :])
            for t in range(2):
                nc.sync.dma_start(bq_sb[t], bq[128 * t:128 * (t + 1), :])
                nc.sync.dma_start(bk_sb[t], bk[128 * t:128 * (t + 1), :])
            nc.sync.dma_start(bvb_sb, bvb[:, :])
            nc.sync.dma_start(bob_sb, bob[:, :])
            nc.sync.dma_start(maskt_sb, maskt[:, :])
            nc.sync.dma_start(tri_sb, tri[:, :])
            nc.sync.dma_start(sel_sb, sel[:, :])

            # ---- Q/K projections (transposed layout, 1024-wide) ----
            # QT[wcol, x] = sum_dm W[dm, wcol] * X[x, dm]
            for xc in range(2):
                for src_dram, w_t, b_t, dst in (
                    (xt_q, wq_sb, bq_sb, qt_sb),
                    (xt_k, wk_sb, bk_sb, kt_sb),
                ):
                    xx = [xtp.tile([128, 1024], BF16, tag="xq", name="xq")
                          for _ in range(DM_T)]
                    for dm in range(DM_T):
                        nc.sync.dma_start(
                            xx[dm],
                            src_dram[128 * dm:128 * (dm + 1),
                                     1024 * xc:1024 * (xc + 1)])
                    for wc in range(2):
                        pq = pa.tile([128, 1024], F32, tag="pa", name="pq")
                        for dm in range(DM_T):
                            for hf in range(2):
                                nc.tensor.matmul(
                                    pq[:, 512 * hf:512 * (hf + 1)],
                                    w_t[dm][:, 128 * wc:128 * (wc + 1)],
                                    xx[dm][:, 512 * hf:512 * (hf + 1)],
                                    start=(dm == 0), stop=(dm == DM_T - 1))
                        with nc.allow_low_precision(reason="bf16 attention"):
                            nc.vector.tensor_scalar_add(
                                dst[wc][:, 1024 * xc:1024 * (xc + 1)], pq, b_t[wc])

            # ---- V projection (natural layout + ones column per head) ----
            for xc in range(S_C):
                xv_t = [xtp.tile([128, 512], BF16, tag="xv", name="xv")
                        for _ in range(DM_T)]
                for dm in range(DM_T):
                    nc.sync.dma_start(
                        xv_t[dm],
                        xt_v[128 * dm:128 * (dm + 1), 512 * xc:512 * (xc + 1)])
                for pr in range(2):
                    psv = [pa.tile([128, WCOL], F32, tag="pa", name="pav")
                           for _ in range(2)]
                    for dm in range(DM_T):
                        for x2 in range(2):
                            nc.tensor.matmul(
                                psv[x2],
                                xv_t[dm][:, 128 * (2 * pr + x2):128 * (2 * pr + x2 + 1)],
                                wv_sb[dm], start=(dm == 0), stop=(dm == DM_T - 1))
                    for x2 in range(2):
                        ki = 4 * xc + 2 * pr + x2
                        va3 = vaug[ki].rearrange("p (h x) -> p h x", h=H_LOC)
                        bvb3 = bvb_sb.rearrange("p (h x) -> p h x", h=H_LOC)
                        with nc.allow_low_precision(reason="bf16 attention"):
                            nc.vector.scalar_tensor_tensor(
                                va3[:, :, 0:DH],
                                psv[x2].rearrange("p (h d) -> p h d", h=H_LOC),
                                1.0, bvb3[:, :, 0:DH],
                                op0=mybir.AluOpType.mult, op1=mybir.AluOpType.add)
                            nc.vector.tensor_copy(
                                va3[:, :, DH:DH + 1], bvb3[:, :, DH:DH + 1])

            # W_O loads (needed only for the output projection)
            for i in range(DM_T):
                nc.sync.dma_start(wo_sb[i], w_o[128 * i:128 * (i + 1), :])

            # ---- causal attention, scores transposed [k, q], 1024-wide ----
            # Fully-masked (k > q) 128-col column blocks are skipped in the
            # scores matmul, exp, and z matmul; esb is zeroed there instead.
            # AllToAll is split in two: heads {0,1} (zt tile 0) fire while
            # heads {2,3} still compute.
            a2a_in = [dp.tile([N_CORES * 128, 256], BF16, tag=f"a2a_in{t}",
                              name=f"a2a_in{t}") for t in range(2)]
            a2a_out = [dp.tile([N_CORES * 128, 256], BF16, tag=f"a2a_out{t}",
                               name=f"a2a_out{t}") for t in range(2)]
            zaug = [ep.tile([DH + 1, 1024], BF16, tag=f"zaug{i}", name=f"zaug{i}",
                            bufs=1) for i in range(4)]
            for h in range(H_LOC):
                th, ho = h // 2, 64 * (h % 2)
                for c in range(2):
                    kmax = 8 * c + 8  # k tiles 0..kmax-1 (rest fully masked)
                    psz = pa.tile([DH + 1, 1024], F32, tag="pa", name="psz")
                    for ki in range(kmax):
                        j = ki - 8 * c
                        lo = 128 * j if j > 0 else 0
                        pss = pspool.tile([128, 1024], F32, tag="ps")
                        for s0, s1 in ((lo, 512), (max(lo, 512), 1024)):
                            if s0 >= s1:
                                continue
                            nc.tensor.matmul(
                                pss[:, s0:s1],
                                kt_sb[th][ho:ho + DH, 128 * ki:128 * (ki + 1)],
                                qt_sb[th][ho:ho + DH, 1024 * c + s0:1024 * c + s1],
                                start=True, stop=True)
                        esb = ep.tile([128, 1024], BF16, tag="e")
                        if j >= 0:  # diagonal: triangle on cols [lo, lo+128)
                            nc.vector.tensor_add(
                                pss[:, lo:lo + 128], pss[:, lo:lo + 128], tri_sb)
                            if lo > 0:
                                nc.vector.memset(esb[:, 0:lo], 0.0)
                        nc.scalar.activation(
                            esb[:, lo:1024], pss[:, lo:1024],
                            mybir.ActivationFunctionType.Exp,
                            bias=maskt_sb[:, ki:ki + 1], scale=float(SCALE))
                        for hf in range(2):
                            nc.tensor.matmul(
                                psz[:, 512 * hf:512 * (hf + 1)],
                                vaug[ki][:, (DH + 1) * h:(DH + 1) * (h + 1)],
                                esb[:, 512 * hf:512 * (hf + 1)],
                                start=(ki == 0), stop=(ki == kmax - 1))
                    # stash unnormalized z + denominator; normalize in a
                    # batched pass per zt tile (single wide reciprocal)
                    za = zaug[2 * (h % 2) + c]
                    with nc.allow_low_precision(reason="bf16 attention"):
                        nc.vector.tensor_copy(za, psz)
                    nc.sync.dma_start(se_sb[2 * (h % 2) + c:2 * (h % 2) + c + 1, :],
                                      za[DH:DH + 1, :])
                if h % 2 == 1:
                    ser = wkp.tile([4, 1024], BF16, tag="ser")
                    with nc.allow_low_precision(reason="bf16 attention"):
                        nc.vector.reciprocal(ser, se_sb)
                    for h2 in range(2):
                        ho2 = 64 * h2
                        for c in range(2):
                            rsel = 2 * h2 + c
                            psb = pspool.tile([DH, 1024], F32, tag="ps", name="psb")
                            for hf in range(2):
                                nc.tensor.matmul(
                                    psb[:, 512 * hf:512 * (hf + 1)],
                                    sel_sb[:, DH * rsel:DH * (rsel + 1)],
                                    ser[:, 512 * hf:512 * (hf + 1)],
                                    start=True, stop=True)
                            with nc.allow_low_precision(reason="bf16 attention"):
                                nc.vector.tensor_mul(
                                    zt_sb[th][ho2:ho2 + DH, 1024 * c:1024 * (c + 1)],
                                    zaug[2 * h2 + c][0:DH, :], psb)
                    # zt tile th complete for all q: fire its AllToAll.
                    # My shard j = my 2 heads' z^T for q cols [256j, 256j+256)
                    # of my batch; received slot p = peer p's heads
                    # {4p+2t, 4p+2t+1} for my 256 q rows.
                    for j in range(N_CORES):
                        nc.sync.dma_start(
                            a2a_in[th][128 * j:128 * (j + 1), :],
                            zt_sb[th][:, 256 * j:256 * (j + 1)])
                    nc.gpsimd.collective_compute(
                        "AllToAll", mybir.AluOpType.bypass,
                        replica_groups=[[0, 1, 2, 3, 4, 5, 6, 7]],
                        ins=[a2a_in[th].opt()], outs=[a2a_out[th].opt()])
                    dst = ztf_e if th == 0 else ztf_o
                    for p in range(N_CORES):
                        nc.sync.dma_start(
                            dst[p], a2a_out[th][128 * p:128 * (p + 1), :])

            # ---- output projection: 256 q rows per batch ----
            # ztf_e[p] holds global heads {4p, 4p+1} -> W_O tile 2(p%4);
            # ztf_o[p] holds heads {4p+2, 4p+3} -> W_O tile 2(p%4)+1.
            for bh in range(2):
                for qt in range(2):
                    osb = wkp.tile([128, DM], F32, tag="osb")
                    pso = pa.tile([128, 1024], F32, tag="pa", name="pso")
                    for hf in range(2):
                        for g in range(4):
                            nc.tensor.matmul(
                                pso[:, 512 * hf:512 * (hf + 1)],
                                ztf_e[4 * bh + g][:, 128 * qt:128 * (qt + 1)],
                                wo_sb[2 * g][:, 512 * hf:512 * (hf + 1)],
                                start=(g == 0), stop=False)
                        for g in range(4):
                            nc.tensor.matmul(
                                pso[:, 512 * hf:512 * (hf + 1)],
                                ztf_o[4 * bh + g][:, 128 * qt:128 * (qt + 1)],
                                wo_sb[2 * g + 1][:, 512 * hf:512 * (hf + 1)],
                                start=False, stop=(g == 3))
                    nc.vector.tensor_add(osb, pso, bob_sb)
                    nc.sync.dma_start(
                        out[256 * bh + 128 * qt:256 * bh + 128 * (qt + 1), :], osb)

    nc.finalize()
    return nc


_NC = None


def _get_nc():
    global _NC
    if _NC is None:
        _NC = build_bass()
    return _NC


def make_in_maps(query_input, key_input, value_input, additive_attention_mask,
                 W_Q, W_K, W_V, W_O, b_Q, b_K, b_V, b_O):
    f = np.float32
    bf = ml_dtypes.bfloat16
    tri = np.where(
        np.arange(128, dtype=np.int64)[None, :]
        >= np.arange(128, dtype=np.int64)[:, None],
        f(0.0), f(MASK_VAL)).astype(f)
    bob = np.ascontiguousarray(np.broadcast_to(b_O.astype(f), (128, DM)))
    sel_host = np.zeros((4, 256), ml_dtypes.bfloat16)
    for rr in range(4):
        sel_host[rr, DH * rr:DH * (rr + 1)] = 1.0
    wo = np.ascontiguousarray(W_O.astype(f).reshape(DM, DM)).astype(bf)
    in_maps = []
    for c in range(N_CORES):
        b, rk = c // GROUP, c % GROUP
        hs = slice(H_LOC * rk, H_LOC * (rk + 1))
        wq = np.ascontiguousarray(
            W_Q[hs].astype(f).transpose(1, 0, 2).reshape(DM, WCOL)).astype(bf)
        wk = np.ascontiguousarray(
            W_K[hs].astype(f).transpose(1, 0, 2).reshape(DM, WCOL)).astype(bf)
        wv = np.ascontiguousarray(
            W_V[hs].astype(f).transpose(1, 0, 2).reshape(DM, WCOL)).astype(bf)
        bvb = np.zeros((128, H_LOC * (DH + 1)), ml_dtypes.bfloat16)
        for h in range(H_LOC):
            bvb[:, (DH + 1) * h:(DH + 1) * h + DH] = b_V[H_LOC * rk + h].astype(f)
            bvb[:, (DH + 1) * h + DH] = 1.0
        in_maps.append({
            "xt_q": np.ascontiguousarray(query_input[b].astype(f).T).astype(bf),
            "xt_k": np.ascontiguousarray(key_input[b].astype(f).T).astype(bf),
            "xt_v": np.ascontiguousarray(value_input[b].astype(f).T).astype(bf),
            "w_q": wq, "w_k": wk, "w_v": wv, "w_o": wo,
            "bq": np.ascontiguousarray(b_Q[hs].astype(f).reshape(WCOL, 1)),
            "bk": np.ascontiguousarray(b_K[hs].astype(f).reshape(WCOL, 1)),
            "bvb": bvb, "bob": bob,
            "sel": sel_host,
            "maskt": np.ascontiguousarray(
                additive_attention_mask[b, 0, 0].astype(f).reshape(S_T, 128).T),
            "tri": tri,
        })
    return in_maps


def assemble_output(results):
    out = np.empty((B, S, DM), np.float32)
    for c in range(N_CORES):
        out[0, 256 * c:256 * (c + 1), :] = results[c]["out"][:256]
        out[1, 256 * c:256 * (c + 1), :] = results[c]["out"][256:]
    return out


def kernel(**inputs):
    nc = _get_nc()
    in_maps = make_in_maps(**inputs)
    res = run_bass_kernel_spmd(nc, in_maps, core_ids=list(range(N_CORES)))
    return assemble_output(res.results)


# revision 18
# speedup vs baseline: 1.5671x; 1.0224x over previous
"""Distributed Trainium2 Bass kernel for a full causal attention layer.

Problem: B=2, S=2048, D_MODEL=1024, H=16, D_HEAD=64, causal + additive mask.

Sharding (8 cores): data-parallel over batch (cores 0-3 -> batch 0,
cores 4-7 -> batch 1) x tensor-parallel over heads (4 heads per core).
Each core:
  1. projects Q,K (transposed layout [head*dhead, seq]) and V (natural
     layout, with an extra ones-column per head) for its 4 heads,
  2. computes causal attention scores transposed S^T[k,q] = K @ Q^T,
     exp via ScalarE (additive mask folded in as per-partition bias,
     causal mask via a precomputed triangle tile on diagonal blocks,
     upper-triangle blocks skipped entirely),
  3. z_aug^T[65,q] = V_aug^T @ E accumulated over k tiles; row 64 is the
     softmax denominator. Normalizes via reciprocal + K=1 broadcast
     matmul.
  4. AllToAll within its 4-core group to reshard z^T from (all q, local
     heads) to (local 512 q rows, all 16 heads),
  5. output projection for its 512 q rows -> disjoint output row slices.
Host only transposes/shards inputs and concatenates the 8 output slices.
"""

import os
import sys

import ml_dtypes
import numpy as np

for _p in ("/opt/trn_rl_repo", "/root/.axon_site/_ro/trn_rl_repo"):
    if os.path.isdir(_p) and _p not in sys.path:
        sys.path.insert(0, _p)

import concourse.bass as bass  # noqa: E402
import concourse.mybir as mybir  # noqa: E402
from concourse import bacc  # noqa: E402
from concourse import tile  # noqa: E402
from concourse.bass_utils import run_bass_kernel_spmd  # noqa: E402

F32 = mybir.dt.float32
F32R = mybir.dt.float32r
BF16 = mybir.dt.bfloat16

B, S, DM, H, DH = 2, 2048, 1024, 16, 64
N_CORES = 8
GROUP = 4              # cores per batch group
H_LOC = H // GROUP     # heads per core
WCOL = H_LOC * DH      # 256 projected cols per core
QR = S // GROUP        # 512 q rows owned per core after AllToAll
MASK_VAL = -1.0e5
SCALE = 1.0 / np.sqrt(DH).astype(np.float32)

DM_T = DM // 128       # 8 dmodel k-tiles
S_T = S // 128         # 16 seq 128-tiles
S_C = S // 512         # 4 seq 512-chunks


def build_bass():
    nc = bacc.Bacc("TRN2", target_bir_lowering=False, debug=False,
                   num_devices=N_CORES)

    xt_q = nc.dram_tensor("xt_q", [DM, S], BF16, kind="ExternalInput")
    xt_k = nc.dram_tensor("xt_k", [DM, S], BF16, kind="ExternalInput")
    xt_v = nc.dram_tensor("xt_v", [DM, S], BF16, kind="ExternalInput")
    w_q = nc.dram_tensor("w_q", [DM, WCOL], BF16, kind="ExternalInput")
    w_k = nc.dram_tensor("w_k", [DM, WCOL], BF16, kind="ExternalInput")
    w_v = nc.dram_tensor("w_v", [DM, WCOL], BF16, kind="ExternalInput")
    w_o = nc.dram_tensor("w_o", [DM, DM], BF16, kind="ExternalInput")
    bq = nc.dram_tensor("bq", [WCOL, 1], F32, kind="ExternalInput")
    bk = nc.dram_tensor("bk", [WCOL, 1], F32, kind="ExternalInput")
    bvb = nc.dram_tensor("bvb", [128, H_LOC * (DH + 1)], BF16, kind="ExternalInput")
    bob = nc.dram_tensor("bob", [128, DM], F32, kind="ExternalInput")
    maskt = nc.dram_tensor("maskt", [128, S_T], F32, kind="ExternalInput")
    tri = nc.dram_tensor("tri", [128, 128], F32, kind="ExternalInput")
    sel = nc.dram_tensor("sel", [4, 256], BF16, kind="ExternalInput")
    out = nc.dram_tensor("out", [QR, DM], F32, kind="ExternalOutput")

    with tile.TileContext(nc) as tc:
        with (
            tc.tile_pool(name="persist", bufs=1) as pp,
            tc.tile_pool(name="xts", bufs=8) as xtp,
            tc.tile_pool(name="esb", bufs=6) as ep,
            tc.tile_pool(name="work", bufs=2) as wkp,
            tc.tile_pool(name="pa", bufs=2, space="PSUM") as pa,
            tc.tile_pool(name="ps", bufs=2, space="PSUM") as pspool,
            tc.tile_pool(name="dram", bufs=1, space="DRAM") as dp,
        ):
            # ---- persistent SBUF tiles ----
            wq_sb = [pp.tile([128, WCOL], BF16, tag=f"wq{i}", name=f"wq{i}") for i in range(DM_T)]
            wk_sb = [pp.tile([128, WCOL], BF16, tag=f"wk{i}", name=f"wk{i}") for i in range(DM_T)]
            wv_sb = [pp.tile([128, WCOL], BF16, tag=f"wv{i}", name=f"wv{i}") for i in range(DM_T)]
            wo_sb = [pp.tile([128, DM], BF16, tag=f"wo{i}", name=f"wo{i}") for i in range(DM_T)]
            qt_sb = [pp.tile([128, S], BF16, tag=f"qt{t}", name=f"qt{t}") for t in range(2)]
            kt_sb = [pp.tile([128, S], BF16, tag=f"kt{t}", name=f"kt{t}") for t in range(2)]
            vaug = [pp.tile([128, H_LOC * (DH + 1)], BF16, tag=f"va{k}", name=f"va{k}")
                    for k in range(S_T)]
            zt_sb = [pp.tile([128, S], BF16, tag=f"zt{t}", name=f"zt{t}") for t in range(2)]
            ztf_e = [pp.tile([128, 256], BF16, tag=f"zfe{i}", name=f"zfe{i}")
                     for i in range(N_CORES)]
            ztf_o = [pp.tile([128, 256], BF16, tag=f"zfo{i}", name=f"zfo{i}")
                     for i in range(N_CORES)]
            bq_sb = [pp.tile([128, 1], F32, tag=f"bq{t}", name=f"bq{t}") for t in range(2)]
            bk_sb = [pp.tile([128, 1], F32, tag=f"bk{t}", name=f"bk{t}") for t in range(2)]
            bvb_sb = pp.tile([128, H_LOC * (DH + 1)], BF16, tag="bvb")
            bob_sb = pp.tile([128, DM], F32, tag="bob")
            maskt_sb = pp.tile([128, S_T], F32, tag="maskt")
            tri_sb = pp.tile([128, 128], F32, tag="tri")
            sel_sb = pp.tile([4, 256], BF16, tag="sel")
            se_sb = pp.tile([4, 1024], BF16, tag="se")

            # ---- load constants + Q/K/V weights (W_O deferred) ----
            for i in range(DM_T):
                nc.sync.dma_start(wq_sb[i], w_q[128 * i:128 * (i + 1), :])
                nc.sync.dma_start(wk_sb[i], w_k[128 * i:128 * (i + 1), :])
                nc.sync.dma_start(wv_sb[i], w_v[128 * i:128 * (i + 1), :])
            for t in range(2):
                nc.sync.dma_start(bq_sb[t], bq[128 * t:128 * (t + 1), :])
                nc.sync.dma_start(bk_sb[t], bk[128 * t:128 * (t + 1), :])
            nc.sync.dma_start(bvb_sb, bvb[:, :])
            nc.sync.dma_start(bob_sb, bob[:, :])
            nc.sync.dma_start(maskt_sb, maskt[:, :])
            nc.sync.dma_start(tri_sb, tri[:, :])
            nc.sync.dma_start(sel_sb, sel[:, :])

            # ---- Q/K projections (transposed layout, 1024-wide) ----
            # QT[wcol, x] = sum_dm W[dm, wcol] * X[x, dm]
            for xc in range(2):
                for src_dram, w_t, b_t, dst in (
                    (xt_q, wq_sb, bq_sb, qt_sb),
                    (xt_k, wk_sb, bk_sb, kt_sb),
                ):
                    xx = [xtp.tile([128, 1024], BF16, tag="xq", name="xq")
                          for _ in range(DM_T)]
                    for dm in range(DM_T):
                        nc.sync.dma_start(
                            xx[dm],
                            src_dram[128 * dm:128 * (dm + 1),
                                     1024 * xc:1024 * (xc + 1)])
                    for wc in range(2):
                        pq = pa.tile([128, 1024], F32, tag="pa", name="pq")
                        for dm in range(DM_T):
                            for hf in range(2):
                                nc.tensor.matmul(
                                    pq[:, 512 * hf:512 * (hf + 1)],
                                    w_t[dm][:, 128 * wc:128 * (wc + 1)],
                                    xx[dm][:, 512 * hf:512 * (hf + 1)],
                                    start=(dm == 0), stop=(dm == DM_T - 1))
                        with nc.allow_low_precision(reason="bf16 attention"):
                            nc.vector.tensor_scalar_add(
                                dst[wc][:, 1024 * xc:1024 * (xc + 1)], pq, b_t[wc])

            # ---- V projection (natural layout + ones column per head) ----
            for xc in range(S_C):
                xv_t = [xtp.tile([128, 512], BF16, tag="xv", name="xv")
                        for _ in range(DM_T)]
                for dm in range(DM_T):
                    nc.sync.dma_start(
                        xv_t[dm],
                        xt_v[128 * dm:128 * (dm + 1), 512 * xc:512 * (xc + 1)])
                for pr in range(2):
                    psv = [pa.tile([128, WCOL], F32, tag="pa", name="pav")
                           for _ in range(2)]
                    for dm in range(DM_T):
                        for x2 in range(2):
                            nc.tensor.matmul(
                                psv[x2],
                                xv_t[dm][:, 128 * (2 * pr + x2):128 * (2 * pr + x2 + 1)],
                                wv_sb[dm], start=(dm == 0), stop=(dm == DM_T - 1))
                    for x2 in range(2):
                        ki = 4 * xc + 2 * pr + x2
                        va3 = vaug[ki].rearrange("p (h x) -> p h x", h=H_LOC)
                        bvb3 = bvb_sb.rearrange("p (h x) -> p h x", h=H_LOC)
                        with nc.allow_low_precision(reason="bf16 attention"):
                            nc.vector.scalar_tensor_tensor(
                                va3[:, :, 0:DH],
                                psv[x2].rearrange("p (h d) -> p h d", h=H_LOC),
                                1.0, bvb3[:, :, 0:DH],
                                op0=mybir.AluOpType.mult, op1=mybir.AluOpType.add)
                            nc.vector.tensor_copy(
                                va3[:, :, DH:DH + 1], bvb3[:, :, DH:DH + 1])

            # W_O loads (needed only for the output projection)
            for i in range(DM_T):
                nc.sync.dma_start(wo_sb[i], w_o[128 * i:128 * (i + 1), :])

            # ---- causal attention, scores transposed [k, q], 1024-wide ----
            # Fully-masked (k > q) 128-col column blocks are skipped in the
            # scores matmul, exp, and z matmul; esb is zeroed there instead.
            # AllToAll is split in two: heads {0,1} (zt tile 0) fire while
            # heads {2,3} still compute.
            a2a_in = [dp.tile([N_CORES * 128, 256], BF16, tag=f"a2a_in{t}",
                              name=f"a2a_in{t}") for t in range(2)]
            a2a_out = [dp.tile([N_CORES * 128, 256], BF16, tag=f"a2a_out{t}",
                               name=f"a2a_out{t}") for t in range(2)]
            zaug = [ep.tile([DH + 1, 1024], BF16, tag=f"zaug{i}", name=f"zaug{i}",
                            bufs=1) for i in range(4)]
            for th in range(2):
                hpair = (2 * th, 2 * th + 1)
                for c in range(2):
                    kmax = 8 * c + 8  # k tiles 0..kmax-1 (rest fully masked)
                    psz = [pa.tile([DH + 1, 1024], F32, tag="pa", name=f"psz{h2}")
                           for h2 in range(2)]
                    pend = [None, None]  # software-pipelined z matmuls
                    for ki in range(kmax):
                        j = ki - 8 * c
                        lo = 128 * j if j > 0 else 0
                        pss = [pspool.tile([128, 1024], F32, tag="ps",
                                           name=f"pss{h2}") for h2 in range(2)]
                        # scores: both heads interleaved so the K=64 matmuls
                        # pack into disjoint PE row groups (base 0 / base 64)
                        for s0, s1 in ((lo, 512), (max(lo, 512), 1024)):
                            if s0 >= s1:
                                continue
                            for h2 in range(2):
                                ho = 64 * h2
                                nc.tensor.matmul(
                                    pss[h2][:, s0:s1],
                                    kt_sb[th][ho:ho + DH, 128 * ki:128 * (ki + 1)],
                                    qt_sb[th][ho:ho + DH, 1024 * c + s0:1024 * c + s1],
                                    start=True, stop=True)
                        esbs = []
                        for h2 in range(2):
                            esb = ep.tile([128, 1024], BF16, tag="e", name="esb")
                            if j >= 0:  # diagonal: triangle on cols [lo, lo+128)
                                nc.vector.tensor_add(
                                    pss[h2][:, lo:lo + 128],
                                    pss[h2][:, lo:lo + 128], tri_sb)
                                if lo > 0:
                                    nc.vector.memset(esb[:, 0:lo], 0.0)
                            nc.scalar.activation(
                                esb[:, lo:1024], pss[h2][:, lo:1024],
                                mybir.ActivationFunctionType.Exp,
                                bias=maskt_sb[:, ki:ki + 1], scale=float(SCALE))
                            esbs.append(esb)
                        for h2 in range(2):
                            if pend[h2] is not None:
                                pki, pesb = pend[h2]
                                for hf in range(2):
                                    nc.tensor.matmul(
                                        psz[h2][:, 512 * hf:512 * (hf + 1)],
                                        vaug[pki][:, (DH + 1) * hpair[h2]:
                                                  (DH + 1) * (hpair[h2] + 1)],
                                        pesb[:, 512 * hf:512 * (hf + 1)],
                                        start=(pki == 0), stop=False)
                            pend[h2] = (ki, esbs[h2])
                    for h2 in range(2):
                        pki, pesb = pend[h2]
                        for hf in range(2):
                            nc.tensor.matmul(
                                psz[h2][:, 512 * hf:512 * (hf + 1)],
                                vaug[pki][:, (DH + 1) * hpair[h2]:
                                          (DH + 1) * (hpair[h2] + 1)],
                                pesb[:, 512 * hf:512 * (hf + 1)],
                                start=(pki == 0), stop=True)
                    # stash unnormalized z + denominator; normalize in a
                    # batched pass per zt tile (single wide reciprocal)
                    for h2 in range(2):
                        za = zaug[2 * h2 + c]
                        with nc.allow_low_precision(reason="bf16 attention"):
                            nc.vector.tensor_copy(za, psz[h2])
                        nc.sync.dma_start(se_sb[2 * h2 + c:2 * h2 + c + 1, :],
                                          za[DH:DH + 1, :])
                ser = wkp.tile([4, 1024], BF16, tag="ser")
                with nc.allow_low_precision(reason="bf16 attention"):
                    nc.vector.reciprocal(ser, se_sb)
                for h2 in range(2):
                    ho2 = 64 * h2
                    for c in range(2):
                        rsel = 2 * h2 + c
                        psb = pspool.tile([DH, 1024], F32, tag="ps", name="psb")
                        for hf in range(2):
                            nc.tensor.matmul(
                                psb[:, 512 * hf:512 * (hf + 1)],
                                sel_sb[:, DH * rsel:DH * (rsel + 1)],
                                ser[:, 512 * hf:512 * (hf + 1)],
                                start=True, stop=True)
                        with nc.allow_low_precision(reason="bf16 attention"):
                            nc.vector.tensor_mul(
                                zt_sb[th][ho2:ho2 + DH, 1024 * c:1024 * (c + 1)],
                                zaug[2 * h2 + c][0:DH, :], psb)
                    # zt tile th complete for all q: fire its AllToAll.
                    # My shard j = my 2 heads' z^T for q cols [256j, 256j+256)
                    # of my batch; received slot p = peer p's heads
                    # {4p+2t, 4p+2t+1} for my 256 q rows.
                    for j in range(N_CORES):
                        nc.sync.dma_start(
                            a2a_in[th][128 * j:128 * (j + 1), :],
                            zt_sb[th][:, 256 * j:256 * (j + 1)])
                    nc.gpsimd.collective_compute(
                        "AllToAll", mybir.AluOpType.bypass,
                        replica_groups=[[0, 1, 2, 3, 4, 5, 6, 7]],
                        ins=[a2a_in[th].opt()], outs=[a2a_out[th].opt()])
                    dst = ztf_e if th == 0 else ztf_o
                    for p in range(N_CORES):
                        nc.sync.dma_start(
                            dst[p], a2a_out[th][128 * p:128 * (p + 1), :])

            # ---- output projection: 256 q rows per batch ----
            # ztf_e[p] holds global heads {4p, 4p+1} -> W_O tile 2(p%4);
            # ztf_o[p] holds heads {4p+2, 4p+3} -> W_O tile 2(p%4)+1.
            for bh in range(2):
                for qt in range(2):
                    osb = wkp.tile([128, DM], F32, tag="osb")
                    pso = pa.tile([128, 1024], F32, tag="pa", name="pso")
                    for hf in range(2):
                        for g in range(4):
                            nc.tensor.matmul(
                                pso[:, 512 * hf:512 * (hf + 1)],
                                ztf_e[4 * bh + g][:, 128 * qt:128 * (qt + 1)],
                                wo_sb[2 * g][:, 512 * hf:512 * (hf + 1)],
                                start=(g == 0), stop=False)
                        for g in range(4):
                            nc.tensor.matmul(
                                pso[:, 512 * hf:512 * (hf + 1)],
                                ztf_o[4 * bh + g][:, 128 * qt:128 * (qt + 1)],
                                wo_sb[2 * g + 1][:, 512 * hf:512 * (hf + 1)],
                                start=False, stop=(g == 3))
                    nc.vector.tensor_add(osb, pso, bob_sb)
                    nc.sync.dma_start(
                        out[256 * bh + 128 * qt:256 * bh + 128 * (qt + 1), :], osb)

    nc.finalize()
    return nc


_NC = None


def _get_nc():
    global _NC
    if _NC is None:
        _NC = build_bass()
    return _NC


def make_in_maps(query_input, key_input, value_input, additive_attention_mask,
                 W_Q, W_K, W_V, W_O, b_Q, b_K, b_V, b_O):
    f = np.float32
    bf = ml_dtypes.bfloat16
    tri = np.where(
        np.arange(128, dtype=np.int64)[None, :]
        >= np.arange(128, dtype=np.int64)[:, None],
        f(0.0), f(MASK_VAL)).astype(f)
    bob = np.ascontiguousarray(np.broadcast_to(b_O.astype(f), (128, DM)))
    sel_host = np.zeros((4, 256), ml_dtypes.bfloat16)
    for rr in range(4):
        sel_host[rr, DH * rr:DH * (rr + 1)] = 1.0
    wo = np.ascontiguousarray(W_O.astype(f).reshape(DM, DM)).astype(bf)
    in_maps = []
    for c in range(N_CORES):
        b, rk = c // GROUP, c % GROUP
        hs = slice(H_LOC * rk, H_LOC * (rk + 1))
        wq = np.ascontiguousarray(
            W_Q[hs].astype(f).transpose(1, 0, 2).reshape(DM, WCOL)).astype(bf)
        wk = np.ascontiguousarray(
            W_K[hs].astype(f).transpose(1, 0, 2).reshape(DM, WCOL)).astype(bf)
        wv = np.ascontiguousarray(
            W_V[hs].astype(f).transpose(1, 0, 2).reshape(DM, WCOL)).astype(bf)
        bvb = np.zeros((128, H_LOC * (DH + 1)), ml_dtypes.bfloat16)
        for h in range(H_LOC):
            bvb[:, (DH + 1) * h:(DH + 1) * h + DH] = b_V[H_LOC * rk + h].astype(f)
            bvb[:, (DH + 1) * h + DH] = 1.0
        in_maps.append({
            "xt_q": np.ascontiguousarray(query_input[b].astype(f).T).astype(bf),
            "xt_k": np.ascontiguousarray(key_input[b].astype(f).T).astype(bf),
            "xt_v": np.ascontiguousarray(value_input[b].astype(f).T).astype(bf),
            "w_q": wq, "w_k": wk, "w_v": wv, "w_o": wo,
            "bq": np.ascontiguousarray(b_Q[hs].astype(f).reshape(WCOL, 1)),
            "bk": np.ascontiguousarray(b_K[hs].astype(f).reshape(WCOL, 1)),
            "bvb": bvb, "bob": bob,
            "sel": sel_host,
            "maskt": np.ascontiguousarray(
                additive_attention_mask[b, 0, 0].astype(f).reshape(S_T, 128).T),
            "tri": tri,
        })
    return in_maps


def assemble_output(results):
    out = np.empty((B, S, DM), np.float32)
    for c in range(N_CORES):
        out[0, 256 * c:256 * (c + 1), :] = results[c]["out"][:256]
        out[1, 256 * c:256 * (c + 1), :] = results[c]["out"][256:]
    return out


def kernel(**inputs):
    nc = _get_nc()
    in_maps = make_in_maps(**inputs)
    res = run_bass_kernel_spmd(nc, in_maps, core_ids=list(range(N_CORES)))
    return assemble_output(res.results)


# revision 19
# speedup vs baseline: 1.6402x; 1.0466x over previous
"""Distributed Trainium2 Bass kernel for a full causal attention layer.

Problem: B=2, S=2048, D_MODEL=1024, H=16, D_HEAD=64, causal + additive mask.

Sharding (8 cores): data-parallel over batch (cores 0-3 -> batch 0,
cores 4-7 -> batch 1) x tensor-parallel over heads (4 heads per core).
Each core:
  1. projects Q,K (transposed layout [head*dhead, seq]) and V (natural
     layout, with an extra ones-column per head) for its 4 heads,
  2. computes causal attention scores transposed S^T[k,q] = K @ Q^T,
     exp via ScalarE (additive mask folded in as per-partition bias,
     causal mask via a precomputed triangle tile on diagonal blocks,
     upper-triangle blocks skipped entirely),
  3. z_aug^T[65,q] = V_aug^T @ E accumulated over k tiles; row 64 is the
     softmax denominator. Normalizes via reciprocal + K=1 broadcast
     matmul.
  4. AllToAll within its 4-core group to reshard z^T from (all q, local
     heads) to (local 512 q rows, all 16 heads),
  5. output projection for its 512 q rows -> disjoint output row slices.
Host only transposes/shards inputs and concatenates the 8 output slices.
"""

import os
import sys

import ml_dtypes
import numpy as np

for _p in ("/opt/trn_rl_repo", "/root/.axon_site/_ro/trn_rl_repo"):
    if os.path.isdir(_p) and _p not in sys.path:
        sys.path.insert(0, _p)

import concourse.bass as bass  # noqa: E402
import concourse.mybir as mybir  # noqa: E402
from concourse import bacc  # noqa: E402
from concourse import tile  # noqa: E402
from concourse.bass_utils import run_bass_kernel_spmd  # noqa: E402

F32 = mybir.dt.float32
F32R = mybir.dt.float32r
BF16 = mybir.dt.bfloat16

B, S, DM, H, DH = 2, 2048, 1024, 16, 64
N_CORES = 8
GROUP = 4              # cores per batch group
H_LOC = H // GROUP     # heads per core
WCOL = H_LOC * DH      # 256 projected cols per core
QR = S // GROUP        # 512 q rows owned per core after AllToAll
MASK_VAL = -1.0e5
SCALE = 1.0 / np.sqrt(DH).astype(np.float32)

DM_T = DM // 128       # 8 dmodel k-tiles
S_T = S // 128         # 16 seq 128-tiles
S_C = S // 512         # 4 seq 512-chunks


def build_bass():
    nc = bacc.Bacc("TRN2", target_bir_lowering=False, debug=False,
                   num_devices=N_CORES)

    xt_q = nc.dram_tensor("xt_q", [DM, S], BF16, kind="ExternalInput")
    xt_k = nc.dram_tensor("xt_k", [DM, S], BF16, kind="ExternalInput")
    xt_v = nc.dram_tensor("xt_v", [DM, S], BF16, kind="ExternalInput")
    w_q = nc.dram_tensor("w_q", [DM, WCOL], BF16, kind="ExternalInput")
    w_k = nc.dram_tensor("w_k", [DM, WCOL], BF16, kind="ExternalInput")
    w_v = nc.dram_tensor("w_v", [DM, WCOL], BF16, kind="ExternalInput")
    w_o = nc.dram_tensor("w_o", [DM, DM], BF16, kind="ExternalInput")
    bq = nc.dram_tensor("bq", [WCOL, 1], F32, kind="ExternalInput")
    bk = nc.dram_tensor("bk", [WCOL, 1], F32, kind="ExternalInput")
    bvb = nc.dram_tensor("bvb", [128, H_LOC * (DH + 1)], BF16, kind="ExternalInput")
    bob = nc.dram_tensor("bob", [128, DM], F32, kind="ExternalInput")
    maskt = nc.dram_tensor("maskt", [128, S_T], F32, kind="ExternalInput")
    tri = nc.dram_tensor("tri", [128, 128], F32, kind="ExternalInput")
    sel = nc.dram_tensor("sel", [4, 256], BF16, kind="ExternalInput")
    out = nc.dram_tensor("out", [QR, DM], F32, kind="ExternalOutput")

    with tile.TileContext(nc) as tc:
        with (
            tc.tile_pool(name="persist", bufs=1) as pp,
            tc.tile_pool(name="xts", bufs=8) as xtp,
            tc.tile_pool(name="esb", bufs=6) as ep,
            tc.tile_pool(name="work", bufs=2) as wkp,
            tc.tile_pool(name="pa", bufs=2, space="PSUM") as pa,
            tc.tile_pool(name="ps", bufs=2, space="PSUM") as pspool,
            tc.tile_pool(name="dram", bufs=1, space="DRAM") as dp,
        ):
            # ---- persistent SBUF tiles ----
            wq_sb = [pp.tile([128, WCOL], BF16, tag=f"wq{i}", name=f"wq{i}") for i in range(DM_T)]
            wk_sb = [pp.tile([128, WCOL], BF16, tag=f"wk{i}", name=f"wk{i}") for i in range(DM_T)]
            wv_sb = [pp.tile([128, WCOL], BF16, tag=f"wv{i}", name=f"wv{i}") for i in range(DM_T)]
            wo_sb = [pp.tile([128, DM], BF16, tag=f"wo{i}", name=f"wo{i}") for i in range(DM_T)]
            qt_sb = [pp.tile([128, S], BF16, tag=f"qt{t}", name=f"qt{t}") for t in range(2)]
            kt_sb = [pp.tile([128, S], BF16, tag=f"kt{t}", name=f"kt{t}") for t in range(2)]
            vaug = [pp.tile([128, H_LOC * (DH + 1)], BF16, tag=f"va{k}", name=f"va{k}")
                    for k in range(S_T)]
            zt_sb = [pp.tile([128, S], BF16, tag=f"zt{t}", name=f"zt{t}") for t in range(2)]
            ztf_e = [pp.tile([128, 256], BF16, tag=f"zfe{i}", name=f"zfe{i}")
                     for i in range(N_CORES)]
            ztf_o = [pp.tile([128, 256], BF16, tag=f"zfo{i}", name=f"zfo{i}")
                     for i in range(N_CORES)]
            bq_sb = [pp.tile([128, 1], F32, tag=f"bq{t}", name=f"bq{t}") for t in range(2)]
            bk_sb = [pp.tile([128, 1], F32, tag=f"bk{t}", name=f"bk{t}") for t in range(2)]
            bvb_sb = pp.tile([128, H_LOC * (DH + 1)], BF16, tag="bvb")
            bob_sb = pp.tile([128, DM], F32, tag="bob")
            maskt_sb = pp.tile([128, S_T], F32, tag="maskt")
            tri_sb = pp.tile([128, 128], F32, tag="tri")
            sel_sb = pp.tile([4, 256], BF16, tag="sel")
            se_sb = pp.tile([4, 1024], BF16, tag="se")

            # ---- load constants + Q/K/V weights (W_O deferred) ----
            for i in range(DM_T):
                nc.sync.dma_start(wq_sb[i], w_q[128 * i:128 * (i + 1), :])
                nc.sync.dma_start(wk_sb[i], w_k[128 * i:128 * (i + 1), :])
                nc.sync.dma_start(wv_sb[i], w_v[128 * i:128 * (i + 1), :])
            for t in range(2):
                nc.sync.dma_start(bq_sb[t], bq[128 * t:128 * (t + 1), :])
                nc.sync.dma_start(bk_sb[t], bk[128 * t:128 * (t + 1), :])
            nc.sync.dma_start(bvb_sb, bvb[:, :])
            nc.sync.dma_start(bob_sb, bob[:, :])
            nc.sync.dma_start(maskt_sb, maskt[:, :])
            nc.sync.dma_start(tri_sb, tri[:, :])
            nc.sync.dma_start(sel_sb, sel[:, :])

            # ---- Q/K projections (transposed layout, 1024-wide) ----
            # QT[wcol, x] = sum_dm W[dm, wcol] * X[x, dm]
            for xc in range(2):
                for src_dram, w_t, b_t, dst in (
                    (xt_q, wq_sb, bq_sb, qt_sb),
                    (xt_k, wk_sb, bk_sb, kt_sb),
                ):
                    xx = [xtp.tile([128, 1024], BF16, tag="xq", name="xq")
                          for _ in range(DM_T)]
                    for dm in range(DM_T):
                        nc.sync.dma_start(
                            xx[dm],
                            src_dram[128 * dm:128 * (dm + 1),
                                     1024 * xc:1024 * (xc + 1)])
                    for wc in range(2):
                        pq = pa.tile([128, 1024], F32, tag="pa", name="pq")
                        for dm in range(DM_T):
                            for hf in range(2):
                                nc.tensor.matmul(
                                    pq[:, 512 * hf:512 * (hf + 1)],
                                    w_t[dm][:, 128 * wc:128 * (wc + 1)],
                                    xx[dm][:, 512 * hf:512 * (hf + 1)],
                                    start=(dm == 0), stop=(dm == DM_T - 1))
                        with nc.allow_low_precision(reason="bf16 attention"):
                            nc.vector.tensor_scalar_add(
                                dst[wc][:, 1024 * xc:1024 * (xc + 1)], pq, b_t[wc])

            # ---- V projection (natural layout + ones column per head) ----
            for xc in range(S_C):
                xv_t = [xtp.tile([128, 512], BF16, tag="xv", name="xv")
                        for _ in range(DM_T)]
                for dm in range(DM_T):
                    nc.sync.dma_start(
                        xv_t[dm],
                        xt_v[128 * dm:128 * (dm + 1), 512 * xc:512 * (xc + 1)])
                for pr in range(2):
                    psv = [pa.tile([128, WCOL], F32, tag="pa", name="pav")
                           for _ in range(2)]
                    for dm in range(DM_T):
                        for x2 in range(2):
                            nc.tensor.matmul(
                                psv[x2],
                                xv_t[dm][:, 128 * (2 * pr + x2):128 * (2 * pr + x2 + 1)],
                                wv_sb[dm], start=(dm == 0), stop=(dm == DM_T - 1))
                    for x2 in range(2):
                        ki = 4 * xc + 2 * pr + x2
                        va3 = vaug[ki].rearrange("p (h x) -> p h x", h=H_LOC)
                        bvb3 = bvb_sb.rearrange("p (h x) -> p h x", h=H_LOC)
                        with nc.allow_low_precision(reason="bf16 attention"):
                            nc.vector.scalar_tensor_tensor(
                                va3[:, :, 0:DH],
                                psv[x2].rearrange("p (h d) -> p h d", h=H_LOC),
                                1.0, bvb3[:, :, 0:DH],
                                op0=mybir.AluOpType.mult, op1=mybir.AluOpType.add)
                            nc.vector.tensor_copy(
                                va3[:, :, DH:DH + 1], bvb3[:, :, DH:DH + 1])

            # W_O loads (needed only for the output projection)
            for i in range(DM_T):
                nc.sync.dma_start(wo_sb[i], w_o[128 * i:128 * (i + 1), :])

            # ---- causal attention, scores transposed [k, q], 1024-wide ----
            # Fully-masked (k > q) 128-col column blocks are skipped in the
            # scores matmul, exp, and z matmul; esb is zeroed there instead.
            # AllToAll is split in two: heads {0,1} (zt tile 0) fire while
            # heads {2,3} still compute.
            a2a_in = [dp.tile([N_CORES * 128, 256], BF16, tag=f"a2a_in{t}",
                              name=f"a2a_in{t}") for t in range(2)]
            a2a_out = [dp.tile([N_CORES * 128, 256], BF16, tag=f"a2a_out{t}",
                               name=f"a2a_out{t}") for t in range(2)]
            zaug = [ep.tile([DH + 1, 1024], BF16, tag=f"zaug{i}", name=f"zaug{i}",
                            bufs=1) for i in range(4)]
            for th in range(2):
                hpair = (2 * th, 2 * th + 1)
                for c in range(2):
                    kmax = 8 * c + 8  # k tiles 0..kmax-1 (rest fully masked)
                    psz = [pa.tile([DH + 1, 1024], F32, tag="pa", name=f"psz{h2}")
                           for h2 in range(2)]
                    pend = [None, None]  # software-pipelined z matmuls
                    for ki in range(kmax):
                        j = ki - 8 * c
                        lo = 128 * j if j > 0 else 0
                        pss = [pspool.tile([128, 1024], F32, tag="ps",
                                           name=f"pss{h2}") for h2 in range(2)]
                        # scores: both heads interleaved so the K=64 matmuls
                        # pack into disjoint PE row groups (base 0 / base 64)
                        for s0, s1 in ((lo, 512), (max(lo, 512), 1024)):
                            if s0 >= s1:
                                continue
                            for h2 in range(2):
                                ho = 64 * h2
                                nc.tensor.matmul(
                                    pss[h2][:, s0:s1],
                                    kt_sb[th][ho:ho + DH, 128 * ki:128 * (ki + 1)],
                                    qt_sb[th][ho:ho + DH, 1024 * c + s0:1024 * c + s1],
                                    start=True, stop=True)
                        esbs = []
                        for h2 in range(2):
                            esb = ep.tile([128, 1024], BF16, tag="e", name="esb")
                            if j >= 0:  # diagonal: triangle on cols [lo, lo+128)
                                nc.vector.tensor_add(
                                    pss[h2][:, lo:lo + 128],
                                    pss[h2][:, lo:lo + 128], tri_sb)
                                if lo > 0:
                                    nc.vector.memset(esb[:, 0:lo], 0.0)
                            nc.scalar.activation(
                                esb[:, lo:1024], pss[h2][:, lo:1024],
                                mybir.ActivationFunctionType.Exp,
                                bias=maskt_sb[:, ki:ki + 1], scale=float(SCALE))
                            esbs.append(esb)
                        for h2 in range(2):
                            if pend[h2] is not None:
                                pki, pesb = pend[h2]
                                for hf in range(2):
                                    nc.tensor.matmul(
                                        psz[h2][:, 512 * hf:512 * (hf + 1)],
                                        vaug[pki][:, (DH + 1) * hpair[h2]:
                                                  (DH + 1) * (hpair[h2] + 1)],
                                        pesb[:, 512 * hf:512 * (hf + 1)],
                                        start=(pki == 0), stop=False)
                            pend[h2] = (ki, esbs[h2])
                    for h2 in range(2):
                        pki, pesb = pend[h2]
                        for hf in range(2):
                            nc.tensor.matmul(
                                psz[h2][:, 512 * hf:512 * (hf + 1)],
                                vaug[pki][:, (DH + 1) * hpair[h2]:
                                          (DH + 1) * (hpair[h2] + 1)],
                                pesb[:, 512 * hf:512 * (hf + 1)],
                                start=(pki == 0), stop=True)
                    # stash unnormalized z + denominator; normalize in a
                    # batched pass per zt tile (single wide reciprocal)
                    for h2 in range(2):
                        za = zaug[2 * h2 + c]
                        with nc.allow_low_precision(reason="bf16 attention"):
                            nc.vector.tensor_copy(za, psz[h2])
                        nc.sync.dma_start(se_sb[2 * h2 + c:2 * h2 + c + 1, :],
                                          za[DH:DH + 1, :])
                ser = wkp.tile([4, 1024], BF16, tag="ser")
                with nc.allow_low_precision(reason="bf16 attention"):
                    nc.vector.reciprocal(ser, se_sb)
                for h2 in range(2):
                    ho2 = 64 * h2
                    for c in range(2):
                        rsel = 2 * h2 + c
                        psb = pspool.tile([DH, 1024], F32, tag="ps", name="psb")
                        for hf in range(2):
                            nc.tensor.matmul(
                                psb[:, 512 * hf:512 * (hf + 1)],
                                sel_sb[:, DH * rsel:DH * (rsel + 1)],
                                ser[:, 512 * hf:512 * (hf + 1)],
                                start=True, stop=True)
                        with nc.allow_low_precision(reason="bf16 attention"):
                            nc.vector.tensor_mul(
                                zt_sb[th][ho2:ho2 + DH, 1024 * c:1024 * (c + 1)],
                                zaug[2 * h2 + c][0:DH, :], psb)
                # zt tile th complete for all q: fire its AllToAll.
                # My shard j = my 2 heads' z^T for q cols [256j, 256j+256)
                # of my batch; received slot p = peer p's heads
                # {4p+2t, 4p+2t+1} for my 256 q rows.
                for j in range(N_CORES):
                    nc.sync.dma_start(
                        a2a_in[th][128 * j:128 * (j + 1), :],
                        zt_sb[th][:, 256 * j:256 * (j + 1)])
                nc.gpsimd.collective_compute(
                    "AllToAll", mybir.AluOpType.bypass,
                    replica_groups=[[0, 1, 2, 3, 4, 5, 6, 7]],
                    ins=[a2a_in[th].opt()], outs=[a2a_out[th].opt()])
                dst = ztf_e if th == 0 else ztf_o
                for p in range(N_CORES):
                    nc.sync.dma_start(
                        dst[p], a2a_out[th][128 * p:128 * (p + 1), :])

            # ---- output projection: 256 q rows per batch ----
            # ztf_e[p] holds global heads {4p, 4p+1} -> W_O tile 2(p%4);
            # ztf_o[p] holds heads {4p+2, 4p+3} -> W_O tile 2(p%4)+1.
            for bh in range(2):
                for qt in range(2):
                    osb = wkp.tile([128, DM], F32, tag="osb")
                    pso = pa.tile([128, 1024], F32, tag="pa", name="pso")
                    for hf in range(2):
                        for g in range(4):
                            nc.tensor.matmul(
                                pso[:, 512 * hf:512 * (hf + 1)],
                                ztf_e[4 * bh + g][:, 128 * qt:128 * (qt + 1)],
                                wo_sb[2 * g][:, 512 * hf:512 * (hf + 1)],
                                start=(g == 0), stop=False)
                        for g in range(4):
                            nc.tensor.matmul(
                                pso[:, 512 * hf:512 * (hf + 1)],
                                ztf_o[4 * bh + g][:, 128 * qt:128 * (qt + 1)],
                                wo_sb[2 * g + 1][:, 512 * hf:512 * (hf + 1)],
                                start=False, stop=(g == 3))
                    nc.vector.tensor_add(osb, pso, bob_sb)
                    nc.sync.dma_start(
                        out[256 * bh + 128 * qt:256 * bh + 128 * (qt + 1), :], osb)

    nc.finalize()
    return nc


_NC = None


def _get_nc():
    global _NC
    if _NC is None:
        _NC = build_bass()
    return _NC


def make_in_maps(query_input, key_input, value_input, additive_attention_mask,
                 W_Q, W_K, W_V, W_O, b_Q, b_K, b_V, b_O):
    f = np.float32
    bf = ml_dtypes.bfloat16
    tri = np.where(
        np.arange(128, dtype=np.int64)[None, :]
        >= np.arange(128, dtype=np.int64)[:, None],
        f(0.0), f(MASK_VAL)).astype(f)
    bob = np.ascontiguousarray(np.broadcast_to(b_O.astype(f), (128, DM)))
    sel_host = np.zeros((4, 256), ml_dtypes.bfloat16)
    for rr in range(4):
        sel_host[rr, DH * rr:DH * (rr + 1)] = 1.0
    wo = np.ascontiguousarray(W_O.astype(f).reshape(DM, DM)).astype(bf)
    in_maps = []
    for c in range(N_CORES):
        b, rk = c // GROUP, c % GROUP
        hs = slice(H_LOC * rk, H_LOC * (rk + 1))
        wq = np.ascontiguousarray(
            W_Q[hs].astype(f).transpose(1, 0, 2).reshape(DM, WCOL)).astype(bf)
        wk = np.ascontiguousarray(
            W_K[hs].astype(f).transpose(1, 0, 2).reshape(DM, WCOL)).astype(bf)
        wv = np.ascontiguousarray(
            W_V[hs].astype(f).transpose(1, 0, 2).reshape(DM, WCOL)).astype(bf)
        bvb = np.zeros((128, H_LOC * (DH + 1)), ml_dtypes.bfloat16)
        for h in range(H_LOC):
            bvb[:, (DH + 1) * h:(DH + 1) * h + DH] = b_V[H_LOC * rk + h].astype(f)
            bvb[:, (DH + 1) * h + DH] = 1.0
        in_maps.append({
            "xt_q": np.ascontiguousarray(query_input[b].astype(f).T).astype(bf),
            "xt_k": np.ascontiguousarray(key_input[b].astype(f).T).astype(bf),
            "xt_v": np.ascontiguousarray(value_input[b].astype(f).T).astype(bf),
            "w_q": wq, "w_k": wk, "w_v": wv, "w_o": wo,
            "bq": np.ascontiguousarray(b_Q[hs].astype(f).reshape(WCOL, 1)),
            "bk": np.ascontiguousarray(b_K[hs].astype(f).reshape(WCOL, 1)),
            "bvb": bvb, "bob": bob,
            "sel": sel_host,
            "maskt": np.ascontiguousarray(
                additive_attention_mask[b, 0, 0].astype(f).reshape(S_T, 128).T),
            "tri": tri,
        })
    return in_maps


def assemble_output(results):
    out = np.empty((B, S, DM), np.float32)
    for c in range(N_CORES):
        out[0, 256 * c:256 * (c + 1), :] = results[c]["out"][:256]
        out[1, 256 * c:256 * (c + 1), :] = results[c]["out"][256:]
    return out


def kernel(**inputs):
    nc = _get_nc()
    in_maps = make_in_maps(**inputs)
    res = run_bass_kernel_spmd(nc, in_maps, core_ids=list(range(N_CORES)))
    return assemble_output(res.results)
